# revision 1
# baseline (speedup 1.0000x reference)
"""Trainium2 Bass kernel: single-head causal attention.

Problem: x[4,4096,128]; Q/K/V linear projections (W [in,out] layout, +bias);
scores = QK^T/sqrt(128) with causal mask; softmax; out = P @ V.

Sharding (8 cores = 4 batches x 2): every core runs the SAME program
(SPMD requirement) on different data:
  core (b, h):
    triangle part: queries q in [2048h, 2048h+2048) of batch b attending
        causally to kv rows in the same range (relative causal structure is
        identical for h=0 and h=1).
    rectangle part: queries q in [2048, 4096) of batch b attending to kv rows
        [1024h, 1024h+1024)  (fully valid, no mask, since kv < 2048 <= q).
  Union over both cores of a batch covers the full causal set exactly once.

Softmax is computed WITHOUT max subtraction (scores are ~N(0,1) by
construction: Wq is pre-scaled by 1/sqrt(128) on host, so exp never
overflows), which makes the cross-core merge linear: the host sums
unnormalized outputs o and denominators l, then divides.

Bias handling:
  - bk drops out of softmax entirely (adds a per-query constant to scores).
  - bq is pre-scaled on host and added to Q^T during the PSUM->SBUF copy
    (per-partition scalar add on the vector engine).
  - bv is added on the host after normalization (rows of P sum to 1).

Matmuls run in float32r (TF32-like: fp32 storage, 11-bit mantissa, full PE
rate at moving free dim >= 256). The BIR verifier requires every producer of
an f32r matmul operand to emit f32r (hardware rounds on write); host-side
inputs are pre-rounded with the exact RNE-to-11-bits rule.

Device layouts (per core):
  xTq [128,4096]  x^T columns for this core's 4096 query slots (tri|rect)
  xTk [128,3072]  x^T columns for kv rows (tri 2048 | rect 1024)
  QT = (x@Wq')^T + bq'  [128(e), 4096(q)]   (e on partitions)
  KT = (x@Wk)^T         [128(e), 3072(k)]
  V  = x@Wv    as 24 tiles [128(kv row), 128(e)] packed in [128, 3072]
  Scores are computed TRANSPOSED: ST[k, q] = K Q^T (PSUM), masked on
  diagonal tiles, exp'd on the scalar engine into P~T [k, q] (SBUF).
  AV:  oT[e, q] += V_t^T-matmul-P~T   (accumulated in PSUM over kv tiles)
  l:   l[q]    += ones-matmul-P~T     (PE is the only partition reducer)
Outputs: oT [128, 4096] (transposed, unnormalized), lv [8,512] (denominators
per 512-query chunk). Host transposes, merges, normalizes, adds bv.
"""

import math
import sys

import numpy as np

sys.path.insert(0, "/opt/trn_rl_repo")

import concourse.bass as bass  # noqa: E402
import concourse.mybir as mybir  # noqa: E402
from concourse.tile import TileContext  # noqa: E402

B, T, D = 4, 4096, 128
HALF = T // 2          # 2048 queries per triangle
NCHUNK = 8             # 8 chunks of 512 query slots per core (4 tri + 4 rect)
CHUNK = 512
KV_TRI_TILES = 16      # triangle kv tiles (2048 rows)
KV_RECT_TILES = 8      # rectangle kv tiles (1024 rows)
KV_TILES = KV_TRI_TILES + KV_RECT_TILES          # 24 tiles = 3072 kv rows
NEG = -1.0e5           # additive mask value; exp(NEG) == 0.0 in fp32

F32 = mybir.dt.float32
F32R = mybir.dt.float32r


def round_f32r(a):
    """Exact fp32 -> fp32r rounding (RNE to 11 mantissa bits), matching
    walrus fp32_to_fp32r."""
    u = np.ascontiguousarray(a, np.float32).view(np.uint32)
    add = np.uint32(0x7FF) + ((u >> np.uint32(12)) & np.uint32(1))
    return ((u + add) & np.uint32(0xFFFFF000)).view(np.float32)


def build_nc(legalize=True):
    nc = bass.Bass()

    xtq_d = nc.declare_dram_parameter("xTq", [D, T], F32R, isOutput=False)
    xtk_d = nc.declare_dram_parameter("xTk", [D, KV_TILES * 128], F32R, isOutput=False)
    wq_d = nc.declare_dram_parameter("Wqs", [D, D], F32R, isOutput=False)
    wk_d = nc.declare_dram_parameter("Wk", [D, D], F32R, isOutput=False)
    wv_d = nc.declare_dram_parameter("Wv", [D, D], F32R, isOutput=False)
    bq_d = nc.declare_dram_parameter("bqs", [D], F32, isOutput=False)
    msk_d = nc.declare_dram_parameter("msk", [4, D, CHUNK], F32R, isOutput=False)
    ident_d = nc.declare_dram_parameter("ident", [D, D], F32R, isOutput=False)
    ones_d = nc.declare_dram_parameter("ones", [D, 1], F32R, isOutput=False)

    ot_d = nc.declare_dram_parameter("oT", [D, T], F32, isOutput=True)
    lv_d = nc.declare_dram_parameter("lv", [NCHUNK, CHUNK], F32, isOutput=True)

    with TileContext(nc) as tc:
        with (
            tc.tile_pool(name="big", bufs=1) as big,
            tc.tile_pool(name="small", bufs=1) as small,
        ):
            # ---- resident SBUF tensors: first-consumed DMAs first (the
            # V projection needs wv + xtk chunk 0 before anything else) ----
            wv = small.tile([D, D], F32R)
            nc.sync.dma_start(out=wv, in_=wv_d[:, :])
            xtk = big.tile([D, KV_TILES * 128], F32R)
            nc.sync.dma_start(out=xtk[:, 0:CHUNK], in_=xtk_d[:, 0:CHUNK])
            wk = small.tile([D, D], F32R)
            nc.sync.dma_start(out=wk, in_=wk_d[:, :])
            wq = small.tile([D, D], F32R)
            nc.sync.dma_start(out=wq, in_=wq_d[:, :])
            bq = small.tile([D, 1], F32)
            nc.sync.dma_start(out=bq, in_=bq_d[:].unsqueeze(1))
            ones = small.tile([D, 1], F32R)
            nc.sync.dma_start(out=ones, in_=ones_d[:, :])
            for j in range(1, KV_TILES * 128 // CHUNK):
                sl = slice(j * CHUNK, (j + 1) * CHUNK)
                nc.sync.dma_start(out=xtk[:, sl], in_=xtk_d[:, sl])
            xtq = big.tile([D, T], F32R)
            for j in range(T // 1024):
                sl = slice(j * 1024, (j + 1) * 1024)
                nc.sync.dma_start(out=xtq[:, sl], in_=xtq_d[:, sl])
            ident = small.tile([D, D], F32R)
            nc.sync.dma_start(out=ident, in_=ident_d[:, :])
            msk = big.tile([D, 4 * CHUNK], F32R)
            nc.sync.dma_start(
                out=msk.rearrange("p (m q) -> p m q", m=4),
                in_=msk_d[:, :, :].transpose([1, 0, 2]),
            )

            qt = big.tile([D, T], F32R)               # Q^T (scaled, biased)
            kt = big.tile([D, KV_TILES * 128], F32R)  # K^T
            vsb = big.tile([D, KV_TILES * 128], F32R)  # V tiles [kvrow, e]

            # The ST pool is opened FIRST so the stack allocator gives it
            # PSUM banks the projection phase never touches: the first
            # attention score matmuls then carry no release deps from the
            # projection pools and overlap the projection tail on the PE.
            stp_cm = tc.tile_pool(name="stp", bufs=2, space="PSUM")
            stp = stp_cm.__enter__()
            # ---- projections (order: V, K, Q so the DVE tick PE waits on
            # for qt also covers vsb/kt; "touch" matmuls absorb each DMA
            # semaphore into PE's clock first, because the fused-weight-load
            # fp32r matmul instruction supports only ONE sync wait) ----
            with (
                tc.tile_pool(name="ppsum", bufs=1, space="PSUM")) as ppsum:
                # (the former "touch" matmuls that absorbed DMA semaphores
                # into PE's clock are gone: the post-Tile wait legalizer
                # handles multi-wait instructions directly, and dropping
                # them frees their PSUM bank for a 4-deep projection
                # rotation plus ~2us of PE dispatch)

                # Pool-recycled PSUM tiles hand every accessor of the new
                # tile the old tile's full release deps (PE write + DVE read)
                # - 2 sync waits, over the fused-weight-load fp32r matmul
                # limit of 1. A single persistent 3-bank tile with manual
                # region rotation keeps deps intra-tile: same-engine WAW is
                # program-order (no sem), so each matmul carries only the
                # DVE WAR wait.
                pps = [ppsum.tile([D, CHUNK], F32, name=f"pps{s}")
                       for s in range(4)]
                nps = [0]

                def proj_ps():
                    s = nps[0] % 4
                    nps[0] += 1
                    return pps[s], s

                for g in range(KV_TILES // 4):     # V: 24 tiles, batched 4/bank
                    ps, s = proj_ps()
                    for jj in range(4):
                        t = 4 * g + jj
                        nc.tensor.matmul(
                            ps[:, jj * 128:(jj + 1) * 128],
                            xtk[:, t * 128:(t + 1) * 128], wv,
                            start=True, stop=True, skip_group_check=True,
                        )
                    if g % 2 == 0:
                        nc.vector.tensor_copy(
                            vsb[:, g * CHUNK:(g + 1) * CHUNK], ps)
                    else:
                        nc.scalar.copy(vsb[:, g * CHUNK:(g + 1) * CHUNK], ps)
                for j in range(KV_TILES * 128 // CHUNK):   # K^T: 6 chunks
                    ps, s = proj_ps()
                    nc.tensor.matmul(
                        ps, wk, xtk[:, j * CHUNK:(j + 1) * CHUNK],
                        start=True, stop=True, skip_group_check=True,
                    )
                    if j % 2 == 0:
                        nc.vector.tensor_copy(
                            kt[:, j * CHUNK:(j + 1) * CHUNK], ps)
                    else:
                        nc.scalar.copy(kt[:, j * CHUNK:(j + 1) * CHUNK], ps)
                for j in range(T // CHUNK):        # Q^T: 8 chunks
                    ps, s = proj_ps()
                    nc.tensor.matmul(
                        ps, wq, xtq[:, j * CHUNK:(j + 1) * CHUNK],
                        start=True, stop=True, skip_group_check=True,
                    )
                    if j % 2 == 0:
                        nc.vector.tensor_scalar_add(
                            qt[:, j * CHUNK:(j + 1) * CHUNK], ps, bq)
                    else:
                        nc.scalar.activation(
                            qt[:, j * CHUNK:(j + 1) * CHUNK], ps,
                            mybir.ActivationFunctionType.Identity, bias=bq)
                # final pump: absorb the last DVE copies before attention

            # ---- attention: 8 chunks, kv-tile pairs, software-pipelined ----
            # chunk c covers query slots [512c, 512c+512).
            # tri chunks (0-3): kv tiles 0..4c+3; rect chunks (4-7): 16..23.
            # Pairs are processed in REVERSE kv order so the diagonal
            # (masked) pairs land at chunk starts, where the previous
            # chunk's AV/l matmuls hide the mask-add + exp latency.
            # The AV+l matmuls of unit u are emitted after ST/exp of unit
            # u+1 (skew-1 software pipeline) so PE never waits on ACT.
            # Tri chunks: the 4 diagonal tiles first in ASCENDING m order
            # (so the first AV/l matmul of the chunk covers the full column
            # range with start=True and later sliced matmuls only ever
            # accumulate onto initialized columns), then the full tiles.
            chunk_ts = [list(range(4 * c, 4 * c + 4)) +
                        list(range(0, 4 * c))[::-1] for c in range(4)] + \
                       [list(range(16, 24))[::-1] for _ in range(4)]
            units = []
            for c, ts in enumerate(chunk_ts):
                pairs = [ts[i:i + 2] for i in range(0, len(ts), 2)]
                for pi, pair in enumerate(pairs):
                    units.append((c, ts, pair, pi == len(pairs) - 1))
            with (
                tc.tile_pool(name="op", bufs=2, space="PSUM") as op,
                tc.tile_pool(name="lp", bufs=2, space="PSUM") as lp,
                tc.tile_pool(name="ptp", bufs=1) as ptp,
                tc.tile_pool(name="osb", bufs=8) as osb,
                tc.tile_pool(name="lsb", bufs=8) as lsb,
            ):
                pts = [ptp.tile([D, 2 * CHUNK], F32R, name=f"pt{i}")
                       for i in range(3)]
                npt = [0]
                acc = {}                # chunk -> (po, pl)
                pending = None          # (c, ts, pair, is_last, pt)
                epiq = []               # delayed epilogues [(c, po, pl)]

                def emit_epilogue():
                    c, po, pl = epiq.pop(0)
                    # epilogue copies on ACT (scalar): the PSUM-slot WAR
                    # dependency of a later chunk's first AV matmul then
                    # consolidates onto the ACT semaphore (1-wait limit).
                    # Delayed one pipeline unit so these ACT ops never sit
                    # between an ST matmul and the exp PE is waiting for.
                    qsl = slice(c * CHUNK, (c + 1) * CHUNK)
                    ob = osb.tile([D, CHUNK], F32, tag="ob", name="ob")
                    nc.vector.tensor_copy(ob, po)
                    nc.sync.dma_start(out=ot_d[:, qsl], in_=ob)
                    lb = lsb.tile([1, CHUNK], F32, tag="lb", name="lb")
                    nc.vector.tensor_copy(lb, pl)
                    nc.sync.dma_start(out=lv_d[c:c + 1, :], in_=lb)

                def emit_av(pend):
                    c, ts, pair, is_last, pt, los = pend
                    if c not in acc:
                        acc[c] = (
                            op.tile([D, CHUNK], F32, tag="po", name="po"),
                            lp.tile([1, CHUNK], F32, tag="pl", name="pl"),
                        )
                    po, pl = acc[c]
                    qsl = slice(c * CHUNK, (c + 1) * CHUNK)
                    for i, t in enumerate(pair):
                        lo = los[i]
                        ptc = pt[:, i * CHUNK + lo:(i + 1) * CHUNK]
                        nc.tensor.matmul(
                            po[:, lo:], vsb[:, t * 128:(t + 1) * 128], ptc,
                            start=(t == ts[0]), stop=(t == ts[-1]),
                            skip_group_check=True,
                        )
                        nc.tensor.matmul(
                            pl[0:1, lo:], ones, ptc,
                            start=(t == ts[0]), stop=(t == ts[-1]),
                            skip_group_check=True,
                        )
                    if is_last:
                        epiq.append((c, po, pl))
                        del acc[c]

                for c, ts, pair, is_last in units:
                    if epiq:
                        emit_epilogue()
                    # Diagonal sub-tile m: every score column q' < 128m is
                    # fully masked (q' < 128m <= 128m + k for all k), so the
                    # ST / mask / exp / AV / l work all skip that prefix.
                    # Within the remaining window only the 128-column band
                    # [128m, 128(m+1)) needs the staircase mask.
                    los = [128 * (t - 4 * c) if c < 4 and t >= 4 * c else 0
                           for t in pair]
                    st = stp.tile([D, 2 * CHUNK], F32, tag="st", name="st")
                    for i, t in enumerate(pair):
                        lo = los[i]
                        nc.tensor.matmul(
                            st[:, i * CHUNK + lo:(i + 1) * CHUNK],
                            kt[:, t * 128:(t + 1) * 128],
                            qt[:, c * CHUNK + lo:(c + 1) * CHUNK],
                            start=True, stop=True, skip_group_check=True,
                        )
                        if c < 4 and t >= 4 * c:
                            m = t - 4 * c
                            nc.tensor.matmul(
                                st[:, i * CHUNK + lo:i * CHUNK + lo + 128],
                                ident,
                                msk[:, m * CHUNK + lo:m * CHUNK + lo + 128],
                                start=False, stop=True, skip_group_check=True,
                            )
                    pt = pts[npt[0] % 3]
                    npt[0] += 1
                    if len(pair) == 2 and los[1] > 0:
                        # sliced halves with an uninitialized gap: exp each
                        # half's valid window separately
                        nc.scalar.activation(
                            pt[:, los[0]:CHUNK], st[:, los[0]:CHUNK],
                            mybir.ActivationFunctionType.Exp,
                        )
                        nc.scalar.activation(
                            pt[:, CHUNK + los[1]:], st[:, CHUNK + los[1]:],
                            mybir.ActivationFunctionType.Exp,
                        )
                    else:
                        nc.scalar.activation(
                            pt[:, los[0]:], st[:, los[0]:],
                            mybir.ActivationFunctionType.Exp,
                        )
                    prev, pending = pending, (c, ts, pair, is_last, pt, los)
                    if prev is not None:
                        emit_av(prev)
                emit_av(pending)
                while epiq:
                    emit_epilogue()
            stp_cm.__exit__(None, None, None)

    if legalize:
        _legalize_multiwaits(nc)
    nc.finalize()
    return nc


def _legalize_multiwaits(nc):
    """Hardware instruction structs in this walrus build accept at most ONE
    sync wait. For any instruction left with >= 2 waits after Tile's sem
    assignment, move all but the last wait onto single-wait same-engine
    NoOps inserted right before it. Engines execute in order, so waiting
    earlier on the same engine preserves semantics exactly.
    """
    for fn in nc.m.functions:
        for blk in fn.blocks:
            insts = blk.instructions
            out = []
            for inst in insts:
                si = inst.sync_info
                if si is not None and si.on_wait and len(si.on_wait) >= 2:
                    waits = list(si.on_wait)
                    for w in waits[:-1]:
                        out.append(mybir.InstNoOp(
                            name=nc.get_next_instruction_name(),
                            engine=inst.engine,
                            bass_nofuse=True,
                            sync_info=mybir.SyncInfo(
                                on_wait=[w], on_update=[]),
                        ))
                    inst.sync_info = mybir.SyncInfo(
                        on_wait=[waits[-1]],
                        on_update=list(si.on_update or []))
                out.append(inst)
            insts[:] = out


_NC_CACHE = {}


def get_nc(legalize=True):
    key = ("nc", legalize)
    if key not in _NC_CACHE:
        _NC_CACHE[key] = build_nc(legalize)
    return _NC_CACHE[key]


def make_core_inputs(x, Wq, bq, Wk, bk, Wv, bv):
    """Per-core input maps (host-side sharding). bk is dropped (softmax
    invariance); bv is applied on the host. f32r-consumed inputs are
    pre-rounded to match the hardware's assumed rounding."""
    s = 1.0 / math.sqrt(D)
    wq_s = round_f32r(np.asarray(Wq, np.float32) * s)
    bq_s = (np.asarray(bq, np.float32) * s).astype(np.float32)
    wk = round_f32r(np.asarray(Wk, np.float32))
    wv = round_f32r(np.asarray(Wv, np.float32))

    # diagonal masks: msk[m][k, q'] = 0 if q' >= 128*m + k else NEG
    qp = np.arange(CHUNK)[None, :]
    kk = np.arange(128)[:, None]
    msk = round_f32r(np.stack(
        [np.where(qp >= 128 * m + kk, 0.0, NEG) for m in range(4)]
    ).astype(np.float32)).reshape(4, D, CHUNK)
    ident = np.eye(D, dtype=np.float32)

    ones = np.ones((D, 1), np.float32)

    x = np.asarray(x, dtype=np.float32)
    in_maps = []
    for core in range(8):
        b, h = core // 2, core % 2
        xb = x[b]                                   # [4096, 128]
        tri = xb[h * HALF:(h + 1) * HALF]           # [2048, 128]
        rect_q = xb[HALF:]                          # [2048, 128]
        rect_kv = xb[h * 1024:(h + 1) * 1024]       # [1024, 128]
        xtq = round_f32r(np.ascontiguousarray(
            np.concatenate([tri, rect_q], axis=0).T))     # [128, 4096]
        xtk = round_f32r(np.ascontiguousarray(
            np.concatenate([tri, rect_kv], axis=0).T))    # [128, 3072]
        in_maps.append({
            "xTq": xtq, "xTk": xtk, "Wqs": wq_s, "Wk": wk, "Wv": wv,
            "bqs": bq_s, "msk": msk, "ones": ones, "ident": ident,
        })
    return in_maps


def merge_outputs(results, bv):
    """Gather per-core (oT, lv) into the full [B, T, D] output."""
    bv = np.asarray(bv, dtype=np.float32)
    out = np.empty((B, T, D), np.float32)
    for b in range(B):
        lo, hi = results[2 * b], results[2 * b + 1]
        O = np.zeros((T, D), np.float64)
        L = np.zeros(T, np.float64)
        O[:HALF] += lo["oT"][:, :HALF].T
        L[:HALF] += lo["lv"][0:4].ravel()
        O[HALF:] += hi["oT"][:, :HALF].T
        L[HALF:] += hi["lv"][0:4].ravel()
        O[HALF:] += lo["oT"][:, HALF:].T
        L[HALF:] += lo["lv"][4:8].ravel()
        O[HALF:] += hi["oT"][:, HALF:].T
        L[HALF:] += hi["lv"][4:8].ravel()
        out[b] = (O / L[:, None]).astype(np.float32) + bv
    return out


def run_per_core(nc, in_maps, threads=True):
    """Run the same single-core program on each NeuronCore with its own
    inputs. The multi-core shard_map path in run_bass_via_pjrt stalls under
    this container's axon tunnel; independent single-device dispatches work
    (the cores share no collectives, so per-core dispatch is equivalent)."""
    import jax
    from concourse import bass2jax

    devices = jax.devices()[:len(in_maps)]

    def one(i):
        with jax.default_device(devices[i]):
            return bass2jax.run_bass_via_pjrt(nc, [in_maps[i]], n_cores=1)[0]

    if threads:
        from concurrent.futures import ThreadPoolExecutor
        # warm the compile cache once to avoid 8 racing neuronxcc compiles
        first = one(0)
        with ThreadPoolExecutor(max_workers=7) as ex:
            rest = list(ex.map(one, range(1, len(in_maps))))
        return [first] + rest
    return [one(i) for i in range(len(in_maps))]


def kernel(x, Wq, bq, Wk, bk, Wv, bv, _trace=False):
    from concourse.bass_utils import axon_active, run_bass_kernel_spmd

    nc = get_nc()
    in_maps = make_core_inputs(x, Wq, bq, Wk, bk, Wv, bv)
    if axon_active():
        # This container tunnels devices through axon; the 8-device
        # shard_map dispatch stalls there, so dispatch per-core.
        results = run_per_core(nc, in_maps)
    else:
        # Native /dev/neuron*: the production NrtSession path.
        res = run_bass_kernel_spmd(nc, in_maps, list(range(8)), trace=_trace)
        kernel.last_result = res
        results = res.results
    out = merge_outputs(results, bv)
    return out



# revision 2
# speedup vs baseline: 1.3867x; 1.3867x over previous
"""Trainium2 Bass kernel v3: single-head causal attention, bf16 datapath.

Problem: x[4,4096,128]; Q/K/V linear projections (W [in,out] layout, +bias);
scores = QK^T/sqrt(128) with causal mask; softmax; out = P @ V.

Sharding (8 cores = 4 batches x 2), identical to v2:
  core (b, h):
    triangle: queries [2048h, 2048h+2048) of batch b attending causally
        within the same range.
    rectangle: queries [2048, 4096) attending to kv rows [1024h, 1024h+1024).
  Union over a batch's two cores covers the causal set exactly once.

Softmax without max subtraction (scores ~N(0,1)); cross-core merge is
linear: host sums unnormalized outputs oT and denominators, then divides.

v3 changes vs v2 (cost-model-driven):
  - bf16 everywhere on the datapath (x, W, Q^T, K^T, V, P~, mask) instead of
    fp32r: halves DMA traffic and SBUF; matmul rate identical (1 cyc/row).
  - NO on-device softmax denominator: the PE "ones" matmuls (l) are gone.
    Instead the DVE accumulates each chunk's exp'd probability tiles into
    ptsum[128, 512] (bf16) and the host reduces the 128 kv-lanes. This takes
    ~14us of matmul time off the PE (the former bottleneck) for ~20us of
    otherwise-idle DVE time.
  - projections interleaved with attention chunks: the ACT engine (exp) and
    the DMA start ~2.5us into the kernel instead of after the whole
    projection phase.
  - epilogue: po (PSUM) -> bf16 SBUF copy -> DMA; ptsum DMAs straight from
    SBUF. Outputs oT[D,T] bf16 + lsum[D,T] bf16.

Per-unit steady state (pair of kv tiles x 512 queries):
  PE: ST pair (1024 rows) + AV pair (1024 rows) ~ 854ns @full clock
  ACT: exp [128, <=1024] ~ 1038ns  <- pacer
  DVE: 2 presum adds ~ 654ns (+ proj copies early on)
Engine budgets/core: PE ~37us (incl ~4us p-state ramp), ACT ~35us, DVE ~38us.
"""

import math
import sys

import numpy as np

sys.path.insert(0, "/opt/trn_rl_repo")

import ml_dtypes  # noqa: E402

import concourse.bass as bass  # noqa: E402
import concourse.mybir as mybir  # noqa: E402
from concourse.tile import TileContext  # noqa: E402

B, T, D = 4, 4096, 128
HALF = T // 2          # 2048 queries per triangle
NCHUNK = 8             # 8 chunks of 512 query slots per core (4 tri + 4 rect)
CHUNK = 512
KV_TRI_TILES = 16      # triangle kv tiles (2048 rows)
KV_RECT_TILES = 8      # rectangle kv tiles (1024 rows)
KV_TILES = KV_TRI_TILES + KV_RECT_TILES          # 24 tiles = 3072 kv rows
NEG = -1.0e5           # additive mask value; exp(NEG) == 0.0

F32 = mybir.dt.float32
BF16 = mybir.dt.bfloat16
bfloat16 = ml_dtypes.bfloat16


def build_nc(legalize=True):
    nc = bass.Bass()

    xtq_d = nc.declare_dram_parameter("xTq", [D, T], BF16, isOutput=False)
    xtk_d = nc.declare_dram_parameter("xTk", [D, KV_TILES * 128], BF16, isOutput=False)
    wa_d = nc.declare_dram_parameter("wpackA", [D, 258], BF16, isOutput=False)
    wb_d = nc.declare_dram_parameter("wpackB", [D, 2 * D + 4 * 128], BF16,
                                     isOutput=False)

    ot_d = nc.declare_dram_parameter("oT", [D, T], BF16, isOutput=True)
    ls_d = nc.declare_dram_parameter("lsum", [D, T], BF16, isOutput=True)

    # attention chunk -> kv tile order: diagonal tiles ascending (valid-col
    # prefix logic needs the widest first), then full tiles in reverse so the
    # masked diagonal work lands right after the chunk's projections.
    chunk_ts = [list(range(0, 4 * c))[::-1] +
                list(range(4 * c, 4 * c + 4)) for c in range(4)] + \
               [list(range(16, 24))[::-1] for _ in range(4)]

    with TileContext(nc) as tc:
        with (
            tc.tile_pool(name="sb", bufs=1) as sb,          # resident tensors
            tc.tile_pool(name="stp", bufs=2, space="PSUM") as stp,   # 4 banks
            tc.tile_pool(name="prj", bufs=2, space="PSUM") as prj,   # 2 banks
            tc.tile_pool(name="op", bufs=2, space="PSUM") as op,     # 2 banks
            tc.tile_pool(name="ptp", bufs=1) as ptp,
            tc.tile_pool(name="pts", bufs=2) as pts,
            tc.tile_pool(name="tmpp", bufs=4) as tmpp,
            tc.tile_pool(name="osb", bufs=2) as osb,
        ):
            # ---- resident SBUF tensors; DMAs issued in first-use order ----
            wa = sb.tile([D, 258], BF16)
            nc.sync.dma_start(out=wa, in_=wa_d[:, :])
            xtk = sb.tile([D, KV_TILES * 128], BF16)
            nc.sync.dma_start(out=xtk[:, 0:CHUNK], in_=xtk_d[:, 0:CHUNK])
            xtq = sb.tile([D, T], BF16)
            nc.sync.dma_start(out=xtq[:, 0:CHUNK], in_=xtq_d[:, 0:CHUNK])
            wb = sb.tile([D, 2 * D + 4 * 128], BF16)
            nc.sync.dma_start(out=wb, in_=wb_d[:, :])
            # bulk x, split and ordered by first use
            nc.sync.dma_start(out=xtk[:, CHUNK:2 * CHUNK],
                              in_=xtk_d[:, CHUNK:2 * CHUNK])
            nc.sync.dma_start(out=xtk[:, 2 * CHUNK:4 * CHUNK],
                              in_=xtk_d[:, 2 * CHUNK:4 * CHUNK])
            nc.sync.dma_start(out=xtq[:, CHUNK:4 * CHUNK],
                              in_=xtq_d[:, CHUNK:4 * CHUNK])
            nc.sync.dma_start(out=xtk[:, 4 * CHUNK:], in_=xtk_d[:, 4 * CHUNK:])
            nc.sync.dma_start(out=xtq[:, 4 * CHUNK:], in_=xtq_d[:, 4 * CHUNK:])
            wk = wa[:, 0:D]
            wq = wa[:, D:2 * D]
            bq = wa[:, 2 * D:2 * D + 2].bitcast(F32)
            wv = wb[:, 0:D]
            ident = wb[:, D:2 * D]
            msk = wb[:, 2 * D:]

            qt = sb.tile([D, T], BF16)                # Q^T (scaled, biased)
            kt = sb.tile([D, KV_TILES * 128], BF16)   # K^T
            vsb = sb.tile([D, KV_TILES * 128], BF16)  # V tiles [kvrow, e]

            pt_tiles = [ptp.tile([D, 2 * CHUNK], BF16, name=f"pt{i}")
                        for i in range(3)]
            npt = [0]
            ntmp = [0]

            # ---- emission helpers ----
            def eng_copy(eng, out, in_):
                if eng is nc.scalar:
                    nc.scalar.copy(out, in_)
                else:
                    (eng or nc.vector).tensor_copy(out, in_)

            def proj_K(g, eng=None, half=None):
                for h in ((0, 1) if half is None else (half,)):
                    sl = slice(g * CHUNK + h * 256, g * CHUNK + (h + 1) * 256)
                    ps = prj.tile([D, 256], F32, tag="prj", name="prjk")
                    nc.tensor.matmul(ps, wk, xtk[:, sl],
                                     start=True, stop=True,
                                     skip_group_check=True)
                    eng_copy(eng, kt[:, sl], ps)

            def proj_V(g, eng=None, half=None):
                for h in ((0, 1) if half is None else (half,)):
                    ps = prj.tile([D, 256], F32, tag="prj", name="prjv")
                    for jj in range(2):
                        t = 4 * g + 2 * h + jj
                        nc.tensor.matmul(
                            ps[:, jj * 128:(jj + 1) * 128],
                            xtk[:, t * 128:(t + 1) * 128], wv,
                            start=True, stop=True, skip_group_check=True)
                    sl = slice(g * CHUNK + h * 256, g * CHUNK + (h + 1) * 256)
                    eng_copy(eng, vsb[:, sl], ps)

            def proj_Q(g, eng=None, half=None):
                for h in ((0, 1) if half is None else (half,)):
                    sl = slice(g * CHUNK + h * 256, g * CHUNK + (h + 1) * 256)
                    ps = prj.tile([D, 256], F32, tag="prj", name="prjq")
                    nc.tensor.matmul(ps, wq, xtq[:, sl],
                                     start=True, stop=True,
                                     skip_group_check=True)
                    if eng is nc.scalar:
                        nc.scalar.activation(
                            qt[:, sl], ps,
                            mybir.ActivationFunctionType.Identity, bias=bq)
                    else:
                        nc.vector.tensor_scalar_add(qt[:, sl], ps, bq)

            state = {"pend": None, "acc": {}, "psum": {}, "epi": [],
                     "projplan": {}, "uidx": 0}

            def emit_epilogue(final=False):
                c, po, psum_t = state["epi"].pop(0)
                qsl = slice(c * CHUNK, (c + 1) * CHUNK)
                # lsum is ready at presum-chain end; ship it before po's copy
                nc.sync.dma_start(out=ls_d[:, qsl], in_=psum_t)
                ob = osb.tile([D, CHUNK], BF16, tag="ob", name="ob")
                if final:
                    nc.scalar.copy(ob, po)   # ACT is idle after the last exp
                else:
                    nc.vector.tensor_copy(ob, po)
                nc.sync.dma_start(out=ot_d[:, qsl], in_=ob)

            def emit_av(pend):
                c, ts, pair, is_last, pt, los = pend
                if c not in state["acc"]:
                    state["acc"][c] = op.tile([D, CHUNK], F32, tag="po",
                                              name="po")
                po = state["acc"][c]
                for i, t in enumerate(pair):
                    lo = los[i]
                    ptc = pt[:, i * CHUNK + lo:(i + 1) * CHUNK]
                    nc.tensor.matmul(
                        po[:, lo:], vsb[:, t * 128:(t + 1) * 128], ptc,
                        start=(t == ts[0]), stop=(t == ts[-1]),
                        skip_group_check=True)
                if is_last:
                    state["epi"].append((c, po, state["psum"][c]))
                    del state["acc"][c]
                    del state["psum"][c]

            def emit_unit(c, ts, pair, is_last, ui):
                """ST + mask (PE), exp (ACT), presum (DVE), delayed AV."""
                if state["epi"]:
                    emit_epilogue()
                los = [128 * (t - 4 * c) if c < 4 and t >= 4 * c else 0
                       for t in pair]
                st = stp.tile([D, 2 * CHUNK], F32, tag="st", name="st")
                for i, t in enumerate(pair):
                    lo = los[i]
                    nc.tensor.matmul(
                        st[:, i * CHUNK + lo:(i + 1) * CHUNK],
                        kt[:, t * 128:(t + 1) * 128],
                        qt[:, c * CHUNK + lo:(c + 1) * CHUNK],
                        start=True, stop=True, skip_group_check=True)
                    if c < 4 and t >= 4 * c:
                        m = t - 4 * c
                        nc.tensor.matmul(
                            st[:, i * CHUNK + lo:i * CHUNK + lo + 128],
                            ident,
                            msk[:, m * 128:(m + 1) * 128],
                            start=False, stop=True, skip_group_check=True)
                pt = pt_tiles[npt[0] % 3]
                npt[0] += 1
                if len(pair) == 2 and los[1] > 0:
                    nc.scalar.activation(
                        pt[:, los[0]:CHUNK], st[:, los[0]:CHUNK],
                        mybir.ActivationFunctionType.Exp)
                    nc.scalar.activation(
                        pt[:, CHUNK + los[1]:], st[:, CHUNK + los[1]:],
                        mybir.ActivationFunctionType.Exp)
                else:
                    nc.scalar.activation(
                        pt[:, los[0]:], st[:, los[0]:],
                        mybir.ActivationFunctionType.Exp)

                # ---- denominator presum into ptsum (bf16) ----
                # tree: tmp = ptA + ptB (Pool/DVE alternating), then the
                # short serial chain ptsum += tmp stays on DVE.
                if ui == 0:
                    psum_t = pts.tile([D, CHUNK], BF16, tag="pts",
                                      name="pts")
                    state["psum"][c] = psum_t
                psum_t = state["psum"][c]
                add = mybir.AluOpType.add
                if los[1] > 0:
                    # diagonal unit: small windowed ops straight on DVE
                    if ui == 0:
                        nc.vector.tensor_copy(
                            psum_t[:, 0:los[1]], pt[:, 0:los[1]])
                        nc.vector.tensor_tensor(
                            out=psum_t[:, los[1]:], in0=pt[:, los[1]:CHUNK],
                            in1=pt[:, CHUNK + los[1]:], op=add)
                    else:
                        for i in range(len(pair)):
                            lo = los[i]
                            nc.vector.tensor_tensor(
                                out=psum_t[:, lo:], in0=psum_t[:, lo:],
                                in1=pt[:, i * CHUNK + lo:(i + 1) * CHUNK],
                                op=add)
                elif ui == 0:
                    nc.vector.tensor_tensor(
                        out=psum_t, in0=pt[:, 0:CHUNK],
                        in1=pt[:, CHUNK:], op=add)
                else:
                    tmp = tmpp.tile([D, CHUNK], BF16, tag="tmp", name="tmp")
                    eng = nc.gpsimd
                    ntmp[0] += 1
                    eng.tensor_tensor(out=tmp, in0=pt[:, 0:CHUNK],
                                      in1=pt[:, CHUNK:], op=add)
                    nc.vector.tensor_tensor(out=psum_t, in0=psum_t,
                                            in1=tmp, op=add)

                for fn in state["projplan"].get(state["uidx"], ()):
                    fn()
                state["uidx"] += 1
                prev, state["pend"] = state["pend"], (c, ts, pair, is_last,
                                                      pt, los)
                if prev is not None:
                    emit_av(prev)

            def emit_chunk(c):
                ts = chunk_ts[c]
                pairs = [ts[i:i + 2] for i in range(0, len(ts), 2)]
                for pi, pair in enumerate(pairs):
                    emit_unit(c, ts, pair, pi == len(pairs) - 1, pi)

            # ---- PE warmup: back-to-back dummy matmuls from t~0.6us keep
            # the tensor engine's p-state ramp running during the input DMA
            # wait, so real projections start at full clock. Operands are
            # uninitialized SBUF (qt is first WRITTEN later); results land in
            # prj tiles that are overwritten with start=True. ----
            scr = sb.tile([D, 384], BF16)
            nc.gpsimd.memset(scr, 0.0)
            for _ in range(14):
                wup = prj.tile([D, CHUNK], F32, tag="prj", name="wup")
                nc.tensor.matmul(wup[:, 0:256], scr[:, 0:D],
                                 scr[:, D:D + 256],
                                 start=True, stop=True, skip_group_check=True)

            # ---- interleaved schedule ----
            # Group-0 projections run up front (qt0's copy rides the idle
            # ACT so kt0 on DVE lands in parallel). Every other projection
            # piece is queued and dropped one-per-unit into the attention
            # stream, ordered by first use. Chunk 3 runs last so the kernel
            # tail is a small masked diagonal unit, not a full one.
            proj_K(0, "split"); proj_Q(0, "split")
            # static plan: unit index -> proj half-pieces to emit there,
            # each ~2-4 units ahead of first use (chunk order 0,1,2,4,5,6,7,3)
            fns = {"Q": proj_Q, "K": proj_K, "V": proj_V}
            plan = {
                0: [("Q", 1, 0), ("Q", 1, 1)],
                1: [("K", 1, 0)],
                2: [("K", 1, 1), ("V", 1, 0)],
                3: [("V", 1, 1), ("Q", 2, 0)],
                4: [("Q", 2, 1)],
                6: [("K", 2, 0)],
                7: [("K", 2, 1), ("V", 2, 0)],
                8: [("V", 2, 1), ("Q", 4, 0)],
                9: [("Q", 4, 1), ("K", 5, 1)],
                10: [("K", 5, 0), ("V", 5, 1)],
                11: [("V", 5, 0), ("K", 4, 1)],
                12: [("K", 4, 0), ("V", 4, 1)],
                13: [("V", 4, 0), ("Q", 5, 0)],
                14: [("Q", 5, 1)],
                16: [("Q", 6, 0)],
                17: [("Q", 6, 1)],
                20: [("Q", 7, 0)],
                21: [("Q", 7, 1)],
                26: [("Q", 3, 0)],
                27: [("Q", 3, 1)],
                28: [("K", 3, 0)],
                29: [("K", 3, 1)],
                30: [("V", 3, 0)],
                31: [("V", 3, 1)],
            }
            state["projplan"] = {
                u: [(lambda f=fns[k], g=g, h=h: f(g, None, h))
                    for k, g, h in pieces]
                for u, pieces in plan.items()
            }
            for c in (0, 1, 2, 4, 5, 6, 7, 3):
                emit_chunk(c)
            emit_av(state["pend"])
            while state["epi"]:
                emit_epilogue(final=len(state["epi"]) == 1)

    if legalize:
        _legalize_multiwaits(nc)
    nc.finalize()
    return nc


def _legalize_multiwaits(nc):
    """Hardware instruction structs accept at most ONE sync wait. Move all
    but the last wait onto single-wait same-engine NoOps inserted before the
    instruction (same-engine program order preserves semantics)."""
    for fn in nc.m.functions:
        for blk in fn.blocks:
            insts = blk.instructions
            out = []
            for inst in insts:
                si = inst.sync_info
                if si is not None and si.on_wait and len(si.on_wait) >= 2:
                    waits = list(si.on_wait)
                    for w in waits[:-1]:
                        out.append(mybir.InstNoOp(
                            name=nc.get_next_instruction_name(),
                            engine=inst.engine,
                            bass_nofuse=True,
                            sync_info=mybir.SyncInfo(
                                on_wait=[w], on_update=[]),
                        ))
                    inst.sync_info = mybir.SyncInfo(
                        on_wait=[waits[-1]],
                        on_update=list(si.on_update or []))
                out.append(inst)
            insts[:] = out


_NC_CACHE = {}


def get_nc(legalize=True):
    key = ("nc", legalize)
    if key not in _NC_CACHE:
        _NC_CACHE[key] = build_nc(legalize)
    return _NC_CACHE[key]


def make_core_inputs(x, Wq, bq, Wk, bk, Wv, bv):
    """Per-core input maps (host-side sharding). bk drops out of softmax;
    bv is applied on the host."""
    s = 1.0 / math.sqrt(D)
    wq_s = (np.asarray(Wq, np.float32) * s).astype(bfloat16)
    bq_s = (np.asarray(bq, np.float32) * s)
    wk = np.asarray(Wk, np.float32).astype(bfloat16)
    wv = np.asarray(Wv, np.float32).astype(bfloat16)

    qp = np.arange(128)[None, :]
    kk = np.arange(128)[:, None]
    # per-m staircase band (query cols [128m, 128m+128) relative part)
    msk = np.concatenate(
        [np.where(qp >= kk, 0.0, NEG) for m in range(4)],
        axis=1).astype(bfloat16)
    ident = np.eye(D, dtype=np.float32).astype(bfloat16)
    bq_bits = np.ascontiguousarray(bq_s[:, None]).view(bfloat16)
    wpackA = np.concatenate([wk, wq_s, bq_bits], axis=1)
    wpackB = np.concatenate([wv, ident, msk], axis=1)

    x = np.asarray(x, dtype=np.float32)
    in_maps = []
    for core in range(8):
        b, h = core // 2, core % 2
        xb = x[b]                                   # [4096, 128]
        tri = xb[h * HALF:(h + 1) * HALF]           # [2048, 128]
        rect_q = xb[HALF:]                          # [2048, 128]
        rect_kv = xb[h * 1024:(h + 1) * 1024]       # [1024, 128]
        xtq = np.ascontiguousarray(
            np.concatenate([tri, rect_q], axis=0).T).astype(bfloat16)
        xtk = np.ascontiguousarray(
            np.concatenate([tri, rect_kv], axis=0).T).astype(bfloat16)
        in_maps.append({
            "xTq": xtq, "xTk": xtk, "wpackA": wpackA, "wpackB": wpackB,
        })
    return in_maps


def merge_outputs(results, bv):
    """Gather per-core (oT, lsum) into the full [B, T, D] output."""
    bv = np.asarray(bv, dtype=np.float32)
    out = np.empty((B, T, D), np.float32)
    for b in range(B):
        lo, hi = results[2 * b], results[2 * b + 1]
        lo_oT = np.asarray(lo["oT"], np.float64)
        hi_oT = np.asarray(hi["oT"], np.float64)
        lo_l = np.asarray(lo["lsum"], np.float64).sum(axis=0)   # [T]
        hi_l = np.asarray(hi["lsum"], np.float64).sum(axis=0)   # [T]
        O = np.zeros((T, D), np.float64)
        L = np.zeros(T, np.float64)
        O[:HALF] += lo_oT[:, :HALF].T
        L[:HALF] += lo_l[:HALF]
        O[HALF:] += hi_oT[:, :HALF].T
        L[HALF:] += hi_l[:HALF]
        O[HALF:] += lo_oT[:, HALF:].T
        L[HALF:] += lo_l[HALF:]
        O[HALF:] += hi_oT[:, HALF:].T
        L[HALF:] += hi_l[HALF:]
        out[b] = (O / L[:, None]).astype(np.float32) + bv
    return out


def run_per_core(nc, in_maps, threads=True):
    """Run the same single-core program on each NeuronCore with its own
    inputs (independent dispatch; the cores share no collectives)."""
    import jax
    from concourse import bass2jax

    devices = jax.devices()[:len(in_maps)]

    def one(i):
        with jax.default_device(devices[i]):
            return bass2jax.run_bass_via_pjrt(nc, [in_maps[i]], n_cores=1)[0]

    if threads:
        from concurrent.futures import ThreadPoolExecutor
        first = one(0)
        with ThreadPoolExecutor(max_workers=7) as ex:
            rest = list(ex.map(one, range(1, len(in_maps))))
        return [first] + rest
    return [one(i) for i in range(len(in_maps))]


def kernel(x, Wq, bq, Wk, bk, Wv, bv, _trace=False):
    from concourse.bass_utils import axon_active, run_bass_kernel_spmd

    nc = get_nc()
    in_maps = make_core_inputs(x, Wq, bq, Wk, bk, Wv, bv)
    if axon_active():
        results = run_per_core(nc, in_maps)
    else:
        res = run_bass_kernel_spmd(nc, in_maps, list(range(8)), trace=_trace)
        kernel.last_result = res
        results = res.results
    out = merge_outputs(results, bv)
    return out
